# revision 1
# baseline (speedup 1.0000x reference)
"""GAT layer (nn_GATLayer) Trainium2 Bass kernel, 8-core SPMD row-sharded.

Math (per core, rows m0..m0+1024 of the 8192-node graph):
  h  = X @ W                      (full h computed on every core; cheap)
  s1 = h @ a1, s2 = h @ a2
  e[m, j]   = leaky_relu(s1[m] + s2[j], 0.2)
  att_u     = where(A > 0, exp(e), 0)        # softmax w/o max-subtraction
  out[m, :] = (att_u @ h) / (att_u @ 1)      # ones-column trick on PE

Layout: everything transposed [j, m] so the PE contraction dim j sits on
partitions. The A row-slab is staged host-side as a contiguous transpose
(still int32, same HBM bytes) so A^T tiles load with plain DMAs.
"""

import os
import sys

sys.path.insert(0, "/opt/trn_rl_repo")

import numpy as np

import concourse.bacc as bacc
import concourse.bass as bass
import concourse.tile as tile
from concourse import mybir
from concourse.bass_utils import run_bass_kernel_spmd

N, FIN, FOUT = 8192, 128, 64
NCORES = 8
MLOC = N // NCORES          # 1024 rows per core
NT = N // 128               # 64 j-tiles
MT = MLOC // 128            # 8 m-tiles
CHUNK = int(os.environ.get("GAT_CHUNK", "4"))  # j-tiles per pipeline chunk
ALPHA = 0.2

F32 = mybir.dt.float32
I32 = mybir.dt.int32
AF = mybir.ActivationFunctionType


def build_kernel():
    nc = bacc.Bacc("TRN2", target_bir_lowering=False)

    x_d = nc.dram_tensor("x_full", (N, FIN), F32, kind="ExternalInput")
    xl_d = nc.dram_tensor("x_loc", (MLOC, FIN), F32, kind="ExternalInput")
    at_d = nc.dram_tensor("at_slab", (N, MLOC), I32, kind="ExternalInput")
    w_d = nc.dram_tensor("w", (FIN, FOUT), F32, kind="ExternalInput")
    a12_d = nc.dram_tensor("a12", (FOUT, 2), F32, kind="ExternalInput")
    out_d = nc.dram_tensor("out", (MLOC, FOUT), F32, kind="ExternalOutput")
    eye_d = nc.inline_tensor(np.eye(128, dtype=np.float32), "eye128")

    with tile.TileContext(nc) as tc:
        with (
            tc.tile_pool(name="const", bufs=1) as constp,
            tc.tile_pool(name="persist", bufs=1) as pers,
            tc.tile_pool(name="xchunk", bufs=2) as xchp,
            tc.tile_pool(name="xt", bufs=3) as xtp,
            tc.tile_pool(name="echunk",
                         bufs=int(os.environ.get("GAT_EBUFS", "3"))) as ep,
            tc.tile_pool(name="at",
                         bufs=int(os.environ.get("GAT_ABUFS", "3"))) as atp,
            tc.tile_pool(name="small", bufs=4) as smp,
            tc.tile_pool(name="outp", bufs=3) as outp,
            tc.tile_pool(name="psA", bufs=2, space="PSUM") as psA,
            tc.tile_pool(name="psO", bufs=1, space="PSUM") as psO,
            tc.tile_pool(name="psC", bufs=2, space="PSUM") as psC,
        ):
            # ---------------- constants ----------------
            eye = constp.tile([128, 128], F32)
            nc.sync.dma_start(out=eye, in_=eye_d.ap())
            w_sb = constp.tile([128, FOUT], F32)
            nc.sync.dma_start(out=w_sb, in_=w_d.ap())
            a12_sb = constp.tile([FOUT, 2], F32)
            nc.sync.dma_start(out=a12_sb, in_=a12_d.ap())

            # ---------------- w1/w2 = W @ a1, W @ a2 ----------------
            wt_ps = psA.tile([FOUT, 128], F32, tag="tr")
            nc.tensor.transpose(wt_ps, w_sb, eye)          # W^T [64, 128]
            wt_sb = constp.tile([FOUT, 128], F32)
            nc.vector.tensor_copy(wt_sb, wt_ps)
            w12_ps = psA.tile([128, 2], F32, tag="h")
            nc.tensor.matmul(w12_ps, lhsT=wt_sb, rhs=a12_sb)   # [c, 2]
            w12_sb = constp.tile([128, 2], F32)
            nc.vector.tensor_copy(w12_sb, w12_ps)

            # Wext = [W | w1 | w2]  (rhs of the h matmuls)
            wext = constp.tile([128, FOUT + 2], F32)
            nc.vector.tensor_copy(wext[:, 0:FOUT], w_sb)
            nc.vector.tensor_copy(wext[:, FOUT:FOUT + 2], w12_sb)

            # ---------------- H' tiles + s1/s2 columns ----------------
            # hp[:, jt, 0:64] = h tile jt, hp[:, jt, 64] = 1.0 (ones column)
            hp = pers.tile([128, NT, FOUT + 1], F32)
            nc.vector.memset(hp[:, :, FOUT:FOUT + 1], 1.0)
            s12 = pers.tile([128, NT, 2], F32)

            x_view = x_d.ap().rearrange("(t p) c -> p t c", p=128)  # [128, 64, 128]
            for g in range(NT // CHUNK):
                xch = xchp.tile([128, CHUNK, FIN], F32)
                nc.sync.dma_start(out=xch, in_=x_view[:, g * CHUNK:(g + 1) * CHUNK, :])
                for k in range(CHUNK):
                    jt = g * CHUNK + k
                    xt_ps = psA.tile([128, 128], F32, tag="tr")
                    nc.tensor.transpose(xt_ps, xch[:, k, :], eye)   # X^T tile [c, i]
                    xt_sb = xtp.tile([128, 128], F32)
                    nc.vector.tensor_copy(xt_sb, xt_ps)
                    h_ps = psA.tile([128, FOUT + 2], F32, tag="h")
                    nc.tensor.matmul(h_ps, lhsT=xt_sb, rhs=wext)    # [i, h|s1|s2]
                    nc.vector.tensor_copy(hp[:, jt, 0:FOUT], h_ps[:, 0:FOUT])
                    nc.vector.tensor_copy(s12[:, jt, :], h_ps[:, FOUT:FOUT + 2])

            # ---------------- s1 replicated across partitions ----------------
            xl_view = xl_d.ap().rearrange("(t p) c -> p t c", p=128)  # [128, 8, 128]
            xln = pers.tile([128, MT, FIN], F32)
            nc.sync.dma_start(out=xln, in_=xl_view)
            xlt = pers.tile([128, MT, 128], F32)
            for t in range(MT):
                xlt_ps = psA.tile([128, 128], F32, tag="tr")
                nc.tensor.transpose(xlt_ps, xln[:, t, :], eye)
                nc.vector.tensor_copy(xlt[:, t, :], xlt_ps)
            # s1_row [1, 1024] = w1^T @ Xloc^T
            s1row = constp.tile([1, MLOC], F32)
            for half in range(2):
                s1row_ps = psA.tile([1, 512], F32, tag="h")
                nc.tensor.matmul(
                    s1row_ps,
                    lhsT=w12_sb[:, 0:1],
                    rhs=xlt.rearrange("p t c -> p (t c)")[:, half * 512:(half + 1) * 512],
                )
                nc.vector.tensor_copy(s1row[:, half * 512:(half + 1) * 512], s1row_ps)
            ones1 = constp.tile([1, 128], F32)
            nc.vector.memset(ones1, 1.0)
            s1rep = pers.tile([128, MLOC], F32)
            for half in range(2):
                rep_ps = psA.tile([128, 512], F32, tag="tr")
                nc.tensor.matmul(
                    rep_ps, lhsT=ones1,
                    rhs=s1row[:, half * 512:(half + 1) * 512],
                )
                nc.vector.tensor_copy(s1rep[:, half * 512:(half + 1) * 512], rep_ps)

            # ---------------- main loop over j-tiles ----------------
            # GAT_REPEAT > 1 repeats the (idempotent) accumulation for
            # slope-based timing; output is unchanged.
            repeat = int(os.environ.get("GAT_REPEAT", "1"))
            ablate = set(os.environ.get("GAT_ABLATE", "").split(","))
            out_ps = psO.tile([FOUT + 1, MLOC], F32)   # [65, 1024] accumulator
            for _rep in range(repeat):
              def consume(g, ech, at4):
                  # mask (two j-tiles per DVE op) then accumulate on PE
                  jt0 = g * CHUNK
                  for q in range(max(1, CHUNK // 2)):
                      k2 = 2 * q
                      w = min(2, CHUNK - k2)
                      esl2 = ech[:, k2 * MLOC:(k2 + w) * MLOC]
                      if "nott" not in ablate:
                          if "nodma" in ablate:
                              nc.vector.tensor_mul(esl2, esl2, esl2)
                          else:
                              nc.vector.tensor_mul(
                                  esl2, esl2,
                                  at4.rearrange("p a m -> p (a m)")[
                                      :, k2 * MLOC:(k2 + w) * MLOC])
                  if "nomm" in ablate and g > 0:
                      return
                  last_jt = CHUNK - 1 if "nomm" in ablate else NT - 1
                  for q in range(CHUNK):
                      jt = jt0 + q
                      esl = ech[:, q * MLOC:(q + 1) * MLOC]
                      for half in range(2):
                          nc.tensor.matmul(
                              out_ps[:, half * 512:(half + 1) * 512],
                              lhsT=hp[:, jt, :],
                              rhs=esl[:, half * 512:(half + 1) * 512],
                              start=(jt == 0),
                              stop=(jt == last_jt),
                          )

              pending = None
              for g in range(NT // CHUNK):
                  ech = ep.tile([128, CHUNK * MLOC], F32)
                  at4 = None
                  if "nodma" not in ablate:
                      at4 = atp.tile([128, CHUNK, MLOC], I32)
                      src = bass.AP(
                          tensor=at_d, offset=g * CHUNK * 128 * MLOC,
                          ap=[[MLOC, 128], [128 * MLOC, CHUNK], [1, MLOC]],
                      )
                      nc.sync.dma_start(out=at4, in_=src)
                  if "noact" not in ablate:
                      for k in range(CHUNK):
                          jt = g * CHUNK + k
                          # e = leaky_relu(s1[m] + s2[j]); s2 rides the bias slot
                          nc.scalar.activation(
                              ech[:, k * MLOC:(k + 1) * MLOC], s1rep, AF.Prelu,
                              bias=s12[:, jt, 1:2], scale=1.0, alpha=ALPHA)
                      nc.scalar.activation(ech, ech, AF.Exp)
                  else:
                      for k in range(CHUNK):
                          jt = g * CHUNK + k
                          nc.vector.tensor_scalar_add(
                              ech[:, k * MLOC:(k + 1) * MLOC], s1rep,
                              s12[:, jt, 1:2])
                  if pending is not None:
                      consume(*pending)
                  pending = (g, ech, at4)
              consume(*pending)

            # ---------------- finalize: transpose, normalize, store ----------
            ot_sb = pers.tile([FOUT + 1, MLOC], F32)
            nc.vector.tensor_copy(ot_sb, out_ps)
            for mt in range(MT):
                tr_ps = psC.tile([128, FOUT + 1], F32)
                nc.tensor.transpose(
                    tr_ps, ot_sb[:, mt * 128:(mt + 1) * 128],
                    eye[0:FOUT + 1, 0:FOUT + 1],
                )
                rec = smp.tile([128, 1], F32)
                nc.vector.reciprocal(rec, tr_ps[:, FOUT:FOUT + 1])
                fin = outp.tile([128, FOUT], F32)
                nc.vector.tensor_scalar_mul(fin, tr_ps[:, 0:FOUT], rec)
                nc.sync.dma_start(
                    out=out_d.ap()[mt * 128:(mt + 1) * 128, :], in_=fin
                )

    nc.compile()
    return nc


_NC = None


def kernel(X, A, W, a1, a2):
    global _NC
    X = np.ascontiguousarray(np.asarray(X, dtype=np.float32))
    A = np.asarray(A, dtype=np.int32)
    W = np.ascontiguousarray(np.asarray(W, dtype=np.float32))
    a12 = np.ascontiguousarray(
        np.stack([np.asarray(a1, dtype=np.float32),
                  np.asarray(a2, dtype=np.float32)], axis=1)
    )
    if _NC is None:
        _NC = build_kernel()
    nc = _NC
    in_maps = []
    for c in range(NCORES):
        rows = slice(c * MLOC, (c + 1) * MLOC)
        in_maps.append({
            "x_full": X,
            "x_loc": np.ascontiguousarray(X[rows]),
            "at_slab": np.ascontiguousarray(A[rows].T),
            "w": W,
            "a12": a12,
        })
    res = run_bass_kernel_spmd(nc, in_maps, core_ids=list(range(NCORES)))
    return np.concatenate([r["out"] for r in res.results], axis=0)


if __name__ == "__main__":
    rng = np.random.default_rng(0)
    X = rng.standard_normal((N, FIN), dtype=np.float32)
    A = rng.integers(0, 2, (N, N), dtype=np.int32)
    W = (rng.standard_normal((FIN, FOUT), dtype=np.float32) * 0.05)
    a1 = (rng.standard_normal((FOUT,), dtype=np.float32) * 0.05)
    a2 = (rng.standard_normal((FOUT,), dtype=np.float32) * 0.05)
    out = kernel(X=X, A=A, W=W, a1=a1, a2=a2)
    # numpy reference
    h = X @ W
    s1 = h @ a1
    s2 = h @ a2
    e = s1[:, None] + s2[None, :]
    e = np.where(e > 0, e, ALPHA * e)
    att = np.where(A > 0, np.exp(e - e.max(1, keepdims=True)), 0.0)
    att = att / att.sum(1, keepdims=True)
    ref = att @ h
    err = np.abs(out - ref).max() / np.abs(ref).max()
    print("rel err (max-abs scaled):", err)

